# revision 1
# baseline (speedup 1.0000x reference)
"""HardAttention Bass kernel for 8 TRN2 NeuronCores.

reference math (B=32, T=4096, H=256):
  energy[b,t,h] = relu( sum_k cat(hidden,enc)[b,t,k] * attn_w[h,k] + attn_b[h] )
  scores[b,t]   = sum_h energy[b,t,h] * v[h]
  out           = softmax(scores, axis=t)[:, None, :]

Device strategy (data-parallel over B, 4 batches/core):
  * split attn_w into W1 (hidden half) and W2 (encoder half)
  * fold v into W2 and into the per-batch bias q = hidden@W1.T + attn_b
    (valid because v >= 0: relu(x)*v == relu(x*v))
  * per (t-chunk, b): z[h,t] = W2v.T-tiles @ enc_T-tiles (float32r matmuls);
    relu+bias on ACT (h-chunk 0) and DVE tensor_scalar (h-chunk 1) write
    float32r tiles; PE indicator-matmuls reduce them over h into a [4, t]
    PSUM scores tile (accumulated across b and h-chunks, deferred one
    b-group so PE never head-blocks on ACT/DVE); ACT exp with fused
    free-axis accum
  * tail: reciprocal of sum, per-partition scale split ACT/DVE, DMA out
Inputs are laid out on the host: enc is transposed to [b, k, t] so k lands
on SBUF partitions with fully contiguous DMA rows.
"""

from contextlib import ExitStack

import numpy as np

import concourse.bass as bass
import concourse.tile as tile
from concourse import bacc, mybir
from concourse.bass_utils import run_bass_kernel_spmd

B, T, H = 32, 4096, 256
NCORES = 8
BC = B // NCORES            # 4 batches per core
KC = H // 128               # 2 k-chunks
HC = H // 128               # 2 h-chunks
# variable t-chunking: small first chunk so the first matmul starts early,
# small last chunks so the end-of-stream pipeline drain is short
CHUNKS = [512, 512, 1024, 1024, 512, 512]
assert sum(CHUNKS) == T
NCHUNK = len(CHUNKS)

F32 = mybir.dt.float32
F32R = mybir.dt.float32r

_CACHE = {}
LAST_RESULTS = None


def _build():
    if "nc" in _CACHE:
        return _CACHE["nc"]

    nc = bacc.Bacc(None, target_bir_lowering=False)
    enc_d = nc.dram_tensor("enc", [BC, KC, 128, T], F32R, kind="ExternalInput")
    # packed f32r consts: cols [0:512) = w2v lhsT tiles (kc,hc), [512:528) = ind
    wc_d = nc.dram_tensor("wconst", [128, 512 + BC * BC], F32R, kind="ExternalInput")
    qv_d = nc.dram_tensor("qv", [128, BC * HC], F32, kind="ExternalInput")
    out_d = nc.dram_tensor("scores", [BC, T], F32, kind="ExternalOutput")

    AF = mybir.ActivationFunctionType
    ALU = mybir.AluOpType

    with tile.TileContext(nc) as tc, ExitStack() as ctx:
        const = ctx.enter_context(tc.tile_pool(name="const", bufs=1))
        encp = ctx.enter_context(tc.tile_pool(name="encp", bufs=12))
        work = ctx.enter_context(tc.tile_pool(name="work", bufs=4))
        zp = ctx.enter_context(tc.tile_pool(name="zp", bufs=4, space="PSUM"))
        scp = ctx.enter_context(tc.tile_pool(name="scp", bufs=2, space="PSUM"))
        rsp = ctx.enter_context(tc.tile_pool(name="rsp", bufs=8))
        tailp = ctx.enter_context(tc.tile_pool(name="tail", bufs=1))

        wc_sb = const.tile([128, 512 + BC * BC], F32R, tag="wconst")
        nc.scalar.dma_start(wc_sb[:], wc_d[:])
        qv_sb = const.tile([128, BC * HC], F32, tag="qv")
        nc.scalar.dma_start(qv_sb[:], qv_d[:])

        def w2v_ap(kc, hc):
            off = (kc * HC + hc) * 128
            return wc_sb[:, off : off + 128]

        def ind_ap(b):
            off = 512 + b * BC
            return wc_sb[:, off : off + BC]

        exp_sb = tailp.tile([BC, T], F32, tag="exp")
        sums_sb = tailp.tile([BC, NCHUNK], F32, tag="sums")

        # deferred h-reduction matmuls: emitted LAG b-groups behind their
        # producing relu/add so the in-order PE stream never waits on ACT/DVE.
        # exp entries are deferred one step further so they never head-block
        # the in-order ACT queue while their chunk's reductions finish.
        LAG = 3
        queue = []

        def flush_reduce(limit):
            while len(queue) > limit:
                kind, payload = queue.pop(0)
                if kind == "ones":
                    pscq, bq, sq, r0q, r1q, fin = payload
                    nc.tensor.matmul(
                        pscq[:, bass.ts(sq, 512)],
                        ind_ap(bq),
                        r0q[:],
                        start=(bq == 0),
                        stop=False,
                    )
                    nc.tensor.matmul(
                        pscq[:, bass.ts(sq, 512)],
                        ind_ap(bq),
                        r1q[:],
                        start=False,
                        stop=(bq == BC - 1),
                    )
                    if fin is not None:
                        queue.append(("exp", fin))
                else:
                    cq, pscq, toff, csz = payload
                    nc.scalar.activation(
                        exp_sb[:, toff : toff + csz], pscq[:], AF.Exp,
                        accum_out=sums_sb[:, cq : cq + 1],
                    )

        toff = 0
        for chunk, CHUNK in enumerate(CHUNKS):
            NSUB = CHUNK // 512
            psc = scp.tile([BC, CHUNK], F32, tag="psc")
            for b in range(BC):
                enc_t = []
                for kc in range(KC):
                    et = encp.tile([128, CHUNK], F32R, tag="enc")
                    nc.sync.dma_start(
                        et[:], enc_d[b, kc][:, toff : toff + CHUNK]
                    )
                    enc_t.append(et)
                for sub in range(NSUB):
                    zs = []
                    for hc in range(HC):
                        z = zp.tile([128, 512], F32, tag="z")
                        for kc in range(KC):
                            nc.tensor.matmul(
                                z[:],
                                w2v_ap(kc, hc),
                                enc_t[kc][:, bass.ts(sub, 512)],
                                start=(kc == 0),
                                stop=(kc == KC - 1),
                            )
                        zs.append(z)
                    r0 = rsp.tile([128, 512], F32R, tag="r0")
                    nc.scalar.activation(
                        r0[:], zs[0][:], AF.Relu,
                        bias=qv_sb[:, b * HC : b * HC + 1],
                    )
                    r1 = rsp.tile([128, 512], F32R, tag="r1")
                    nc.vector.tensor_scalar(
                        r1[:], zs[1][:],
                        scalar1=qv_sb[:, b * HC + 1 : b * HC + 2],
                        scalar2=0.0,
                        op0=ALU.add,
                        op1=ALU.max,
                    )
                    fin = None
                    if b == BC - 1 and sub == NSUB - 1:
                        fin = (chunk, psc, toff, CHUNK)
                    queue.append(("ones", (psc, b, sub, r0, r1, fin)))
                flush_reduce(LAG * 2)
            toff += CHUNK
        flush_reduce(0)

        stot = tailp.tile([BC, 1], F32, tag="stot")
        nc.vector.tensor_reduce(
            stot[:], sums_sb[:], axis=mybir.AxisListType.X, op=ALU.add
        )
        recip = tailp.tile([BC, 1], F32, tag="recip")
        nc.vector.reciprocal(recip[:], stot[:])
        outs = tailp.tile([BC, T], F32, tag="outs")
        half = T // 2
        nc.vector.tensor_scalar_mul(
            outs[:, :half], exp_sb[:, :half], recip[:]
        )
        nc.scalar.activation(
            outs[:, half:], exp_sb[:, half:], AF.Copy, scale=recip[:]
        )
        nc.sync.dma_start(out_d[:], outs[:])

    nc.compile()
    _CACHE["nc"] = nc
    return nc


def _prep_inputs(hidden, encoder_outputs, attn_w, attn_b, v):
    w1 = attn_w[:, :H]
    w2 = attn_w[:, H:]
    qv_full = (((hidden @ w1.T) + attn_b) * v).astype(np.float32)   # [B, H]
    w2v = (w2 * v[:, None]).astype(np.float32)     # [H(h), H(k)]
    w2v_T = np.ascontiguousarray(w2v.T)            # [k, h]

    # packed const block: [128, 512+16]
    wconst = np.zeros((128, 512 + BC * BC), dtype=np.float32)
    for kc in range(KC):
        for hc in range(HC):
            off = (kc * HC + hc) * 128
            wconst[:, off : off + 128] = w2v_T[
                kc * 128 : (kc + 1) * 128, hc * 128 : (hc + 1) * 128
            ]
    for b in range(BC):
        wconst[:, 512 + b * BC + b] = 1.0

    in_maps = []
    for c in range(NCORES):
        bs = c * BC
        enc_c = np.ascontiguousarray(
            encoder_outputs[:, bs : bs + BC, :].transpose(1, 2, 0)
        ).reshape(BC, KC, 128, T)
        qv_c = np.ascontiguousarray(
            qv_full[bs : bs + BC].reshape(BC, HC, 128).transpose(2, 0, 1)
        ).reshape(128, BC * HC)
        in_maps.append({"enc": enc_c, "wconst": wconst, "qv": qv_c})
    return in_maps


def kernel(hidden, encoder_outputs, attn_w, attn_b, v):
    global LAST_RESULTS
    nc = _build()
    in_maps = _prep_inputs(
        np.asarray(hidden, dtype=np.float32),
        np.asarray(encoder_outputs, dtype=np.float32),
        np.asarray(attn_w, dtype=np.float32),
        np.asarray(attn_b, dtype=np.float32),
        np.asarray(v, dtype=np.float32),
    )
    res = run_bass_kernel_spmd(nc, in_maps, list(range(NCORES)))
    LAST_RESULTS = res
    out = np.empty((B, 1, T), dtype=np.float32)
    for c in range(NCORES):
        out[c * BC : (c + 1) * BC, 0, :] = res.results[c]["scores"]
    return out



# revision 6
# speedup vs baseline: 1.1185x; 1.1185x over previous
"""HardAttention Bass kernel for 8 TRN2 NeuronCores.

reference math (B=32, T=4096, H=256):
  energy[b,t,h] = relu( sum_k cat(hidden,enc)[b,t,k] * attn_w[h,k] + attn_b[h] )
  scores[b,t]   = sum_h energy[b,t,h] * v[h]
  out           = softmax(scores, axis=t)[:, None, :]

Device strategy (data-parallel over B, 4 batches/core):
  * split attn_w into W1 (hidden half) and W2 (encoder half)
  * fold v into W2 and into the per-batch bias q = hidden@W1.T + attn_b
    (valid because v >= 0: relu(x)*v == relu(x*v))
  * per (t-chunk, b): z[h,t] = W2v.T-tiles @ enc_T-tiles (float32r matmuls);
    relu+bias on ACT (h-chunk 0) and DVE tensor_scalar (h-chunk 1) write
    float32r tiles; PE indicator-matmuls reduce them over h into a [4, t]
    PSUM scores tile (accumulated across b and h-chunks, deferred one
    b-group so PE never head-blocks on ACT/DVE); ACT exp with fused
    free-axis accum
  * tail: reciprocal of sum, per-partition scale split ACT/DVE, DMA out
Inputs are laid out on the host: enc is transposed to [b, k, t] so k lands
on SBUF partitions with fully contiguous DMA rows.
"""

from contextlib import ExitStack

import numpy as np

import concourse.bass as bass
import concourse.tile as tile
from concourse import bacc, mybir
from concourse.bass_utils import run_bass_kernel_spmd

B, T, H = 32, 4096, 256
NCORES = 8
BC = B // NCORES            # 4 batches per core
KC = H // 128               # 2 k-chunks
HC = H // 128               # 2 h-chunks
# variable t-chunking: small first chunk so the first matmul starts early,
# small last chunks so the end-of-stream pipeline drain is short
CHUNKS = [512, 512, 1024, 1024, 512, 512]
assert sum(CHUNKS) == T
NCHUNK = len(CHUNKS)

F32 = mybir.dt.float32
F32R = mybir.dt.float32r
F16 = mybir.dt.float16

_CACHE = {}
LAST_RESULTS = None


def _build():
    if "nc" in _CACHE:
        return _CACHE["nc"]

    nc = bacc.Bacc(None, target_bir_lowering=False)
    enc_d = nc.dram_tensor("enc", [BC, KC, 128, T], F16, kind="ExternalInput")
    # fp16 w2v lhsT tiles (kc,hc)
    w16_d = nc.dram_tensor("w16", [128, 512], F16, kind="ExternalInput")
    # f32r indicator columns for the h-reduction matmuls
    ind_d = nc.dram_tensor("ind", [128, BC * BC], F32R, kind="ExternalInput")
    qv_d = nc.dram_tensor("qv", [128, BC * HC], F32, kind="ExternalInput")
    out_d = nc.dram_tensor("scores", [BC, T], F32, kind="ExternalOutput")

    AF = mybir.ActivationFunctionType
    ALU = mybir.AluOpType

    with tile.TileContext(nc) as tc, ExitStack() as ctx:
        const = ctx.enter_context(tc.tile_pool(name="const", bufs=1))
        encp = ctx.enter_context(tc.tile_pool(name="encp", bufs=12))
        work = ctx.enter_context(tc.tile_pool(name="work", bufs=4))
        zp = ctx.enter_context(tc.tile_pool(name="zp", bufs=4, space="PSUM"))
        scp = ctx.enter_context(tc.tile_pool(name="scp", bufs=2, space="PSUM"))
        rsp = ctx.enter_context(tc.tile_pool(name="rsp", bufs=8))
        tailp = ctx.enter_context(tc.tile_pool(name="tail", bufs=1))

        w16_sb = const.tile([128, 512], F16, tag="w16")
        nc.scalar.dma_start(w16_sb[:], w16_d[:])
        ind_sb = const.tile([128, BC * BC], F32R, tag="ind")
        nc.scalar.dma_start(ind_sb[:], ind_d[:])
        qv_sb = const.tile([128, BC * HC], F32, tag="qv")
        nc.scalar.dma_start(qv_sb[:], qv_d[:])

        def w2v_ap(kc, hc):
            off = (kc * HC + hc) * 128
            return w16_sb[:, off : off + 128]

        def ind_ap(b):
            off = b * BC
            return ind_sb[:, off : off + BC]

        exp_sb = tailp.tile([BC, T], F32, tag="exp")
        sums_sb = tailp.tile([BC, NCHUNK], F32, tag="sums")

        # deferred h-reduction matmuls: emitted LAG b-groups behind their
        # producing relu/add so the in-order PE stream never waits on ACT/DVE.
        # exp entries are deferred one step further so they never head-block
        # the in-order ACT queue while their chunk's reductions finish.
        LAG = 3
        queue = []

        def flush_reduce(limit):
            while len(queue) > limit:
                kind, payload = queue.pop(0)
                if kind == "ones":
                    pscq, bq, sq, r0q, r1q, fin = payload
                    nc.tensor.matmul(
                        pscq[:, bass.ts(sq, 512)],
                        ind_ap(bq),
                        r0q[:],
                        start=(bq == 0),
                        stop=False,
                    )
                    nc.tensor.matmul(
                        pscq[:, bass.ts(sq, 512)],
                        ind_ap(bq),
                        r1q[:],
                        start=False,
                        stop=(bq == BC - 1),
                    )
                    if fin is not None:
                        queue.append(("exp", fin))
                else:
                    cq, pscq, toff, csz = payload
                    nc.scalar.activation(
                        exp_sb[:, toff : toff + csz], pscq[:], AF.Exp,
                        accum_out=sums_sb[:, cq : cq + 1],
                    )

        toff = 0
        for chunk, CHUNK in enumerate(CHUNKS):
            NSUB = CHUNK // 512
            psc = scp.tile([BC, CHUNK], F32, tag="psc")
            for b in range(BC):
                enc_t = []
                for kc in range(KC):
                    et = encp.tile([128, CHUNK], F16, tag="enc")
                    nc.sync.dma_start(
                        et[:], enc_d[b, kc][:, toff : toff + CHUNK]
                    )
                    enc_t.append(et)
                for sub in range(NSUB):
                    zs = []
                    for hc in range(HC):
                        z = zp.tile([128, 512], F32, tag="z")
                        for kc in range(KC):
                            nc.tensor.matmul(
                                z[:],
                                w2v_ap(kc, hc),
                                enc_t[kc][:, bass.ts(sub, 512)],
                                start=(kc == 0),
                                stop=(kc == KC - 1),
                            )
                        zs.append(z)
                    r0 = rsp.tile([128, 512], F32R, tag="r0")
                    nc.scalar.activation(
                        r0[:], zs[0][:], AF.Relu,
                        bias=qv_sb[:, b * HC : b * HC + 1],
                    )
                    r1 = rsp.tile([128, 512], F32R, tag="r1")
                    nc.vector.tensor_scalar(
                        r1[:], zs[1][:],
                        scalar1=qv_sb[:, b * HC + 1 : b * HC + 2],
                        scalar2=0.0,
                        op0=ALU.add,
                        op1=ALU.max,
                    )
                    fin = None
                    if b == BC - 1 and sub == NSUB - 1:
                        fin = (chunk, psc, toff, CHUNK)
                    queue.append(("ones", (psc, b, sub, r0, r1, fin)))
                flush_reduce(LAG * 2)
            toff += CHUNK
        flush_reduce(0)

        stot = tailp.tile([BC, 1], F32, tag="stot")
        nc.vector.tensor_reduce(
            stot[:], sums_sb[:], axis=mybir.AxisListType.X, op=ALU.add
        )
        recip = tailp.tile([BC, 1], F32, tag="recip")
        nc.vector.reciprocal(recip[:], stot[:])
        outs = tailp.tile([BC, T], F32, tag="outs")
        half = T // 2
        nc.vector.tensor_scalar_mul(
            outs[:, :half], exp_sb[:, :half], recip[:]
        )
        nc.scalar.activation(
            outs[:, half:], exp_sb[:, half:], AF.Copy, scale=recip[:]
        )
        nc.sync.dma_start(out_d[:], outs[:])

    nc.compile()
    _CACHE["nc"] = nc
    return nc


def _prep_inputs(hidden, encoder_outputs, attn_w, attn_b, v):
    w1 = attn_w[:, :H]
    w2 = attn_w[:, H:]
    qv_full = (((hidden @ w1.T) + attn_b) * v).astype(np.float32)   # [B, H]
    w2v = (w2 * v[:, None]).astype(np.float32)     # [H(h), H(k)]
    w2v_T = np.ascontiguousarray(w2v.T)            # [k, h]

    # fp16 w2v lhsT tiles: [128, 512]
    w16 = np.zeros((128, 512), dtype=np.float16)
    for kc in range(KC):
        for hc in range(HC):
            off = (kc * HC + hc) * 128
            w16[:, off : off + 128] = w2v_T[
                kc * 128 : (kc + 1) * 128, hc * 128 : (hc + 1) * 128
            ].astype(np.float16)
    ind = np.zeros((128, BC * BC), dtype=np.float32)
    for b in range(BC):
        ind[:, b * BC + b] = 1.0

    enc16 = encoder_outputs.astype(np.float16)     # [T, B, H]
    in_maps = []
    for c in range(NCORES):
        bs = c * BC
        enc_c = np.ascontiguousarray(
            enc16[:, bs : bs + BC, :].transpose(1, 2, 0)
        ).reshape(BC, KC, 128, T)
        qv_c = np.ascontiguousarray(
            qv_full[bs : bs + BC].reshape(BC, HC, 128).transpose(2, 0, 1)
        ).reshape(128, BC * HC)
        in_maps.append({"enc": enc_c, "w16": w16, "ind": ind, "qv": qv_c})
    return in_maps


def kernel(hidden, encoder_outputs, attn_w, attn_b, v):
    global LAST_RESULTS
    nc = _build()
    in_maps = _prep_inputs(
        np.asarray(hidden, dtype=np.float32),
        np.asarray(encoder_outputs, dtype=np.float32),
        np.asarray(attn_w, dtype=np.float32),
        np.asarray(attn_b, dtype=np.float32),
        np.asarray(v, dtype=np.float32),
    )
    res = run_bass_kernel_spmd(nc, in_maps, list(range(NCORES)))
    LAST_RESULTS = res
    out = np.empty((B, 1, T), dtype=np.float32)
    for c in range(NCORES):
        out[c * BC : (c + 1) * BC, 0, :] = res.results[c]["scores"]
    return out



# revision 33
# speedup vs baseline: 1.4853x; 1.3279x over previous
"""HardAttention Bass kernel for 8 TRN2 NeuronCores.

reference math (B=32, T=4096, H=256):
  energy[b,t,h] = relu( sum_k cat(hidden,enc)[b,t,k] * attn_w[h,k] + attn_b[h] )
  scores[b,t]   = sum_h energy[b,t,h] * v[h]
  out           = softmax(scores, axis=t)[:, None, :]

Device strategy (data-parallel over B, 4 batches/core):
  * split attn_w into W1 (hidden half) and W2 (encoder half)
  * fold v into W2 and into the per-batch bias q = hidden@W1.T + attn_b
    (valid because v >= 0: relu(x)*v == relu(x*v))
  * enc streamed as fp16 [k, t] tiles (k on partitions); per (b, 512-col
    sub): z[h,t] = W2v-tiles @ enc-tiles (fp16 matmuls, fp32 psum)
  * relu+bias fused per h-chunk: ACT activation / DVE tensor_scalar,
    writing fp16 r tiles; DVE folds the two h-chunks (fp16 2x add);
    Pool (gpsimd) reduces the 128 h-partitions -> scores row [1, 512]
  * scores live as [32, 512] (row = b*8+sub): exp+accum on ACT in one
    [32,512] op, per-b sums and reciprocal broadcast via tiny PE
    indicator matmuls, final scale on DVE (2x sbuf mode), one DMA out
Host reassembles [32,512] -> [4, 4096] per core (pure reshape).
"""

from contextlib import ExitStack

import numpy as np

import concourse.bass as bass
import concourse.tile as tile
from concourse import bacc, mybir
from concourse.bass_utils import run_bass_kernel_spmd

B, T, H = 32, 4096, 256
NCORES = 8
BC = B // NCORES            # 4 batches per core
KC = H // 128               # 2 k-chunks
HC = H // 128               # 2 h-chunks
SUB = 512                   # t columns per reduction unit
NSUB = T // SUB             # 8 subs per batch
DMAW = 2048                 # t columns per enc DMA
NDMA = T // DMAW            # 2 DMA halves per (b, kc)

F32 = mybir.dt.float32
F16 = mybir.dt.float16

_CACHE = {}
LAST_RESULTS = None


def _build():
    if "nc" in _CACHE:
        return _CACHE["nc"]

    nc = bacc.Bacc(None, target_bir_lowering=False)
    enc_d = nc.dram_tensor("enc", [BC, KC, 128, T], F16, kind="ExternalInput")
    # c16: cols 0:512 = w2v lhsT tiles; cols 512:576 = ind64 (8 blocks of
    # [128,8] with column j of block j all-ones, for b3 PE reductions)
    c16_d = nc.dram_tensor("c16", [128, 512 + 64], F16, kind="ExternalInput")
    # c32: cols 0:8 = qv (per-(b,hc) bias columns); cols 8:24 rows 0:12 = bc
    # (block-ones(4) over b0..b2 rows, and 8x4 all-ones for the b3 total)
    c32_d = nc.dram_tensor("c32", [128, 32], F32, kind="ExternalInput")
    out_d = nc.dram_tensor("scores", [32, SUB], F32, kind="ExternalOutput")
    sc_d = nc.dram_tensor("sc_scratch", [12, 1024], F32, kind="Internal")

    AF = mybir.ActivationFunctionType
    ALU = mybir.AluOpType

    NP = NSUB // 2          # 4 sub-pairs per batch
    PW = 2 * SUB            # 1024 columns per pair

    # engine for each hc1 relu by (b, sub); hc0 relus always run on ACT
    HC1 = {}
    for b in range(BC):
        for s in range(NSUB):
            HC1[(b, s)] = "dve"
    for b in range(BC):
        for s in [0, 4]:
            HC1[(b, s)] = "act"

    with tile.TileContext(nc) as tc, ExitStack() as ctx:
        const = ctx.enter_context(tc.tile_pool(name="const", bufs=1))
        encp = ctx.enter_context(tc.tile_pool(name="encp", bufs=1))
        zp = ctx.enter_context(tc.tile_pool(name="zp", bufs=6, space="PSUM"))
        rsp = ctx.enter_context(tc.tile_pool(name="rsp", bufs=6))
        b3r = ctx.enter_context(tc.tile_pool(name="b3r", bufs=1))
        tailp = ctx.enter_context(tc.tile_pool(name="tail", bufs=1))
        pscp = ctx.enter_context(tc.tile_pool(name="pscp", bufs=1, space="PSUM"))

        # fp16 consts in one DMA on the SP queue ahead of the enc stream;
        # fp32 consts follow the first small enc tiles
        c16_sb = const.tile([128, 512 + 64], F16, tag="c16")
        nc.sync.dma_start(c16_sb[:], c16_d[:])
        c32_sb = const.tile([128, 32], F32, tag="c32")
        w16_sb = c16_sb
        qv_sb = c32_sb

        def w2v_ap(kc, hc):
            off = (kc * HC + hc) * 128
            return w16_sb[:, off : off + 128]

        def ind_ap(j):
            return c16_sb[:, 512 + 8 * j : 512 + 8 * j + 8]

        def bc_ap(rows, cols):
            return c32_sb[rows, 8 + cols.start : 8 + cols.stop]

        # stream in all enc tiles (resident: 8.4 MB total); b0's first 512
        # columns ship as small DMAs so PE starts early
        enc_t = {}
        first = {}
        for kc in range(KC):
            ft = encp.tile([128, SUB], F16, tag=f"enc_f_{kc}")
            nc.gpsimd.dma_start(ft[:], enc_d[0, kc][:, 0:SUB])
            first[kc] = ft
        nc.sync.dma_start(c32_sb[:], c32_d[:])
        for b in range(BC):
            for half in range(NDMA):
                for kc in range(KC):
                    et = encp.tile([128, DMAW], F16, tag=f"enc_{b}_{kc}_{half}")
                    nc.sync.dma_start(
                        et[:], enc_d[b, kc][:, half * DMAW : (half + 1) * DMAW]
                    )
                    enc_t[(b, kc, half)] = et

        scores12 = tailp.tile([12, PW], F32, tag="scores12")
        sall = tailp.tile([1, 12 * PW], F32, tag="sall")
        exp12t = tailp.tile([12, PW], F32, tag="exp12t")
        sums12 = tailp.tile([12, 1], F32, tag="sums12")
        psc8 = pscp.tile([8, SUB], F32, tag="psc8")
        r01_b3 = []
        _rows_done = set()

        def do_pair(b, p):
            r0p = rsp.tile([128, PW], F16, tag="r0p")
            r1p = rsp.tile([128, PW], F16, tag="r1p")
            for s in range(2):
                sub = 2 * p + s
                half, col = divmod(sub * SUB, DMAW)
                zs = []
                for hc in range(HC):
                    z = zp.tile([128, SUB], F32, tag="z")
                    for kc in range(KC):
                        if b == 0 and sub == 0:
                            rhs = first[kc][:]
                        else:
                            rhs = enc_t[(b, kc, half)][:, col : col + SUB]
                        nc.tensor.matmul(
                            z[:],
                            w2v_ap(kc, hc),
                            rhs,
                            start=(kc == 0),
                            stop=(kc == KC - 1),
                        )
                    zs.append(z)
                cols = slice(s * SUB, (s + 1) * SUB)
                nc.scalar.activation(
                    r0p[:, cols], zs[0][:], AF.Relu,
                    bias=qv_sb[:, b * HC : b * HC + 1],
                )
                eng = HC1[(b, 2 * p + s)] if False else HC1.get((b, sub), "dve")
                if eng == "act":
                    nc.scalar.activation(
                        r1p[:, cols], zs[1][:], AF.Relu,
                        bias=qv_sb[:, b * HC + 1 : b * HC + 2],
                    )
                else:
                    nc.vector.tensor_scalar(
                        r1p[:, cols], zs[1][:],
                        scalar1=qv_sb[:, b * HC + 1 : b * HC + 2],
                        scalar2=0.0,
                        op0=ALU.add,
                        op1=ALU.max,
                    )
            if b == BC - 1:
                r01p = b3r.tile([128, PW], F16, tag=f"r01b3_{p}")
            else:
                r01p = rsp.tile([128, PW], F16, tag="r01p")
            nc.vector.tensor_tensor(r01p[:], r0p[:], r1p[:], op=ALU.add)
            if b < BC - 1:
                row = b * NP + p
                nc.gpsimd.tensor_reduce(
                    sall[:, row * PW : (row + 1) * PW], r01p[:],
                    axis=mybir.AxisListType.C, op=ALU.add,
                )
            else:
                r01_b3.append(r01p)

        def b3_red(p):
            # psc8 row 2p+s <- sum_h of pair p's half s (sub order)
            for s in range(2):
                j = 2 * p + s
                nc.tensor.matmul(
                    psc8[:], ind_ap(j),
                    r01_b3[p][:, s * SUB : (s + 1) * SUB],
                    start=(p == 0 and s == 0),
                    stop=(p == NP - 1 and s == 1),
                )

        for b in range(BC - 1):
            for p in range(NP):
                do_pair(b, p)

        # b3 main compute; the b0..b2 tail ops are emitted at stream
        # positions where their deps are already satisfied, and the b3
        # reduction matmuls lag one pair behind the fold that feeds them
        do_pair(BC - 1, 0)
        # gather the partition-0 scores strip into [12, PW] via a DRAM
        # bounce (cheap: 48 KB each way, hidden under b3 compute)
        nc.sync.dma_start(sc_d[:], sall[:])
        nc.sync.dma_start(scores12[:], sc_d[:])
        # exp for b0..b2 (ACT stream: lands after b3p0's relus)
        nc.scalar.activation(
            exp12t[:], scores12[:], AF.Exp, accum_out=sums12[:],
        )
        do_pair(BC - 1, 1)
        b3_red(0)
        do_pair(BC - 1, 2)
        b3_red(1)
        do_pair(BC - 1, 3)
        b3_red(2)
        b3_red(3)

        # ---- remaining tail ----
        gs12 = pscp.tile([12, 1], F32, tag="gsx")
        nc.tensor.matmul(gs12[:], c32_sb[0:12, 8:20], sums12[:],
                         start=True, stop=True)
        exp8 = tailp.tile([8, SUB], F32, tag="exp8")
        acc8 = tailp.tile([8, 1], F32, tag="acc8")
        nc.scalar.activation(exp8[:], psc8[:], AF.Exp, accum_out=acc8[:])
        recip12 = tailp.tile([12, 1], F32, tag="recip12")
        nc.vector.reciprocal(recip12[:], gs12[:])
        outs12 = tailp.tile([12, PW], F32, tag="outs12")
        nc.vector.tensor_scalar_mul(outs12[:], exp12t[:], recip12[:])
        nc.sync.dma_start(out_d[0:24], outs12[:])
        gs8 = pscp.tile([8, 1], F32, tag="gsx")
        nc.tensor.matmul(gs8[:], c32_sb[0:8, 24:32], acc8[:],
                         start=True, stop=True)
        recip8 = tailp.tile([8, 1], F32, tag="recip8")
        nc.vector.reciprocal(recip8[:], gs8[:])
        outs8 = tailp.tile([8, SUB], F32, tag="outs8")
        nc.vector.tensor_scalar_mul(outs8[:, 0:256], exp8[:, 0:256], recip8[:])
        nc.sync.dma_start(out_d[24:32, 0:256], outs8[:, 0:256])
        nc.vector.tensor_scalar_mul(outs8[:, 256:SUB], exp8[:, 256:SUB], recip8[:])
        nc.sync.dma_start(out_d[24:32, 256:SUB], outs8[:, 256:SUB])

    nc.compile()
    _CACHE["nc"] = nc
    return nc


def _prep_inputs(hidden, encoder_outputs, attn_w, attn_b, v):
    w1 = attn_w[:, :H]
    w2 = attn_w[:, H:]
    qv_full = (((hidden @ w1.T) + attn_b) * v).astype(np.float32)   # [B, H]
    w2v = (w2 * v[:, None]).astype(np.float32)     # [H(h), H(k)]
    w2v_T = np.ascontiguousarray(w2v.T)            # [k, h]

    w16 = np.zeros((128, KC * HC * 128), dtype=np.float16)
    for kc in range(KC):
        for hc in range(HC):
            off = (kc * HC + hc) * 128
            w16[:, off : off + 128] = w2v_T[
                kc * 128 : (kc + 1) * 128, hc * 128 : (hc + 1) * 128
            ].astype(np.float16)

    c16 = np.zeros((128, 512 + 64), dtype=np.float16)
    c16[:, 0:512] = w16
    for j in range(8):
        c16[:, 512 + 8 * j + j] = 1.0

    enc16 = encoder_outputs.astype(np.float16)     # [T, B, H]
    in_maps = []
    for c in range(NCORES):
        bs = c * BC
        enc_c = np.ascontiguousarray(
            enc16[:, bs : bs + BC, :].transpose(1, 2, 0)
        ).reshape(BC, KC, 128, T)
        qv_c = np.ascontiguousarray(
            qv_full[bs : bs + BC].reshape(BC, HC, 128).transpose(2, 0, 1)
        ).reshape(128, BC * HC)
        c32 = np.zeros((128, 32), dtype=np.float32)
        c32[:, 0:8] = qv_c
        for bb in range(3):
            c32[bb * 4 : (bb + 1) * 4, 8 + bb * 4 : 8 + (bb + 1) * 4] = 1.0
        c32[0:8, 24:32] = 1.0
        in_maps.append(
            {"enc": enc_c, "c16": c16, "c32": c32}
        )
    return in_maps


def kernel(hidden, encoder_outputs, attn_w, attn_b, v):
    global LAST_RESULTS
    nc = _build()
    in_maps = _prep_inputs(
        np.asarray(hidden, dtype=np.float32),
        np.asarray(encoder_outputs, dtype=np.float32),
        np.asarray(attn_w, dtype=np.float32),
        np.asarray(attn_b, dtype=np.float32),
        np.asarray(v, dtype=np.float32),
    )
    res = run_bass_kernel_spmd(nc, in_maps, list(range(NCORES)))
    LAST_RESULTS = res
    out = np.empty((B, 1, T), dtype=np.float32)
    for c in range(NCORES):
        out[c * BC : (c + 1) * BC, 0, :] = (
            res.results[c]["scores"].reshape(BC, T)
        )
    return out


# revision 51
# speedup vs baseline: 1.4931x; 1.0053x over previous
"""HardAttention Bass kernel for 8 TRN2 NeuronCores.

reference math (B=32, T=4096, H=256):
  energy[b,t,h] = relu( sum_k cat(hidden,enc)[b,t,k] * attn_w[h,k] + attn_b[h] )
  scores[b,t]   = sum_h energy[b,t,h] * v[h]
  out           = softmax(scores, axis=t)[:, None, :]

Device strategy (data-parallel over B, 4 batches/core):
  * split attn_w into W1 (hidden half) and W2 (encoder half)
  * fold v into W2 and into the per-batch bias q = hidden@W1.T + attn_b
    (valid because v >= 0: relu(x)*v == relu(x*v))
  * enc streamed as fp16 [k, t] tiles (k on partitions); per (b, 512-col
    sub): z[h,t] = W2v-tiles @ enc-tiles (fp16 matmuls, fp32 psum)
  * relu+bias fused per h-chunk: ACT activation / DVE tensor_scalar,
    writing fp16 r tiles; DVE folds the two h-chunks (fp16 2x add);
    Pool (gpsimd) reduces the 128 h-partitions -> scores row [1, 512]
  * scores live as [32, 512] (row = b*8+sub): exp+accum on ACT in one
    [32,512] op, per-b sums and reciprocal broadcast via tiny PE
    indicator matmuls, final scale on DVE (2x sbuf mode), one DMA out
Host reassembles [32,512] -> [4, 4096] per core (pure reshape).
"""

from contextlib import ExitStack

import numpy as np

import concourse.bass as bass
import concourse.tile as tile
from concourse import bacc, mybir
from concourse.bass_utils import run_bass_kernel_spmd

B, T, H = 32, 4096, 256
NCORES = 8
BC = B // NCORES            # 4 batches per core
KC = H // 128               # 2 k-chunks
HC = H // 128               # 2 h-chunks
SUB = 512                   # t columns per reduction unit
NSUB = T // SUB             # 8 subs per batch
DMAW = 2048                 # t columns per enc DMA
NDMA = T // DMAW            # 2 DMA halves per (b, kc)

F32 = mybir.dt.float32
F16 = mybir.dt.float16

_CACHE = {}
LAST_RESULTS = None


def _build():
    if "nc" in _CACHE:
        return _CACHE["nc"]

    nc = bacc.Bacc(None, target_bir_lowering=False)
    enc_d = nc.dram_tensor("enc", [BC, KC, 128, T], F16, kind="ExternalInput")
    # c16: cols 0:512 = w2v lhsT tiles; cols 512:576 = ind64 (8 blocks of
    # [128,8] with column j of block j all-ones, for b3 PE reductions)
    c16_d = nc.dram_tensor("c16", [128, 512 + 64], F16, kind="ExternalInput")
    # c32: cols 0:8 = qv (per-(b,hc) bias columns); cols 8:24 rows 0:12 = bc
    # (block-ones(4) over b0..b2 rows, and 8x4 all-ones for the b3 total)
    c32_d = nc.dram_tensor("c32", [128, 32], F32, kind="ExternalInput")
    out_d = nc.dram_tensor("scores", [32, SUB], F32, kind="ExternalOutput")
    sc_d = nc.dram_tensor("sc_scratch", [12, 1024], F32, kind="Internal")

    AF = mybir.ActivationFunctionType
    ALU = mybir.AluOpType

    NP = NSUB // 2          # 4 sub-pairs per batch
    PW = 2 * SUB            # 1024 columns per pair

    # engine for each hc1 relu by (b, sub); hc0 relus always run on ACT
    HC1 = {}
    for b in range(BC):
        for s in range(NSUB):
            HC1[(b, s)] = "dve"
    for b in range(BC):
        for s in [0, 4]:
            HC1[(b, s)] = "act"

    with tile.TileContext(nc) as tc, ExitStack() as ctx:
        const = ctx.enter_context(tc.tile_pool(name="const", bufs=1))
        encp = ctx.enter_context(tc.tile_pool(name="encp", bufs=1))
        zp = ctx.enter_context(tc.tile_pool(name="zp", bufs=6, space="PSUM"))
        rsp = ctx.enter_context(tc.tile_pool(name="rsp", bufs=6))
        b3r = ctx.enter_context(tc.tile_pool(name="b3r", bufs=1))
        tailp = ctx.enter_context(tc.tile_pool(name="tail", bufs=1))
        pscp = ctx.enter_context(tc.tile_pool(name="pscp", bufs=1, space="PSUM"))

        # fp16 consts in one DMA on the SP queue ahead of the enc stream;
        # fp32 consts follow the first small enc tiles
        c16_sb = const.tile([128, 512 + 64], F16, tag="c16")
        nc.sync.dma_start(c16_sb[:], c16_d[:])
        c32_sb = const.tile([128, 32], F32, tag="c32")
        w16_sb = c16_sb
        qv_sb = c32_sb


        def w2v_ap(kc, hc):
            off = (kc * HC + hc) * 128
            return w16_sb[:, off : off + 128]

        def ind_ap(j):
            return c16_sb[:, 512 + 8 * j : 512 + 8 * j + 8]

        def bc_ap(rows, cols):
            return c32_sb[rows, 8 + cols.start : 8 + cols.stop]

        # stream in all enc tiles (resident: 8.4 MB total). The first two
        # (b0, half0) tiles are split [0:512]+[512:2048] so PE's first pair
        # only waits on two small transfers.
        enc_t = {}
        first = {}
        for kc in range(KC):
            ft = encp.tile([128, SUB], F16, tag=f"enc_f_{kc}")
            nc.gpsimd.dma_start(ft[:], enc_d[0, kc][:, 0:SUB])
            first[kc] = ft
        nc.sync.dma_start(c32_sb[:], c32_d[:])

        HW = DMAW // 2
        for b in range(BC):
            for half in range(NDMA):
                for kc in range(KC):
                    et = encp.tile([128, DMAW], F16, tag=f"enc_{b}_{kc}_{half}")
                    enc_t[(b, kc, half)] = et
                if b == 0:
                    # finer chunks for the first batch keep PE fed while it
                    # chases the incoming stream
                    for kc in range(KC):
                        for q in range(2):
                            lo = half * DMAW + q * HW
                            nc.sync.dma_start(
                                enc_t[(b, kc, half)][:, q * HW : (q + 1) * HW],
                                enc_d[b, kc][:, lo : lo + HW],
                            )
                else:
                    for kc in range(KC):
                        nc.sync.dma_start(
                            enc_t[(b, kc, half)][:],
                            enc_d[b, kc][:, half * DMAW : (half + 1) * DMAW],
                        )

        scores12 = tailp.tile([12, PW], F32, tag="scores12")
        sall = tailp.tile([1, 12 * PW], F32, tag="sall")
        exp12t = tailp.tile([12, PW], F32, tag="exp12t")
        sums12 = tailp.tile([12, 1], F32, tag="sums12")
        psc8 = pscp.tile([8, SUB], F32, tag="psc8")


        r01_b3 = []
        _rows_done = set()

        def do_pair(b, p):
            r0p = rsp.tile([128, PW], F16, tag="r0p")
            r1p = rsp.tile([128, PW], F16, tag="r1p")
            last = b == BC - 1 and p == NP - 1

            def dve_relu(dst, zsrc, hc):
                nc.vector.tensor_scalar(
                    dst, zsrc,
                    scalar1=qv_sb[:, b * HC + hc : b * HC + hc + 1],
                    scalar2=0.0,
                    op0=ALU.add,
                    op1=ALU.max,
                )

            for s in range(2):
                sub = 2 * p + s
                half, col = divmod(sub * SUB, DMAW)
                zs = []
                for hc in range(HC):
                    z = zp.tile([128, SUB], F32, tag="z")
                    for kc in range(KC):
                        if b == 0 and sub == 0:
                            rhs = first[kc][:]
                        else:
                            rhs = enc_t[(b, kc, half)][:, col : col + SUB]
                        nc.tensor.matmul(
                            z[:],
                            w2v_ap(kc, hc),
                            rhs,
                            start=(kc == 0),
                            stop=(kc == KC - 1),
                        )
                    zs.append(z)
                cols = slice(s * SUB, (s + 1) * SUB)
                nc.scalar.activation(
                    r0p[:, cols], zs[0][:], AF.Relu,
                    bias=qv_sb[:, b * HC : b * HC + 1],
                )
                if HC1.get((b, sub), "dve") == "act":
                    nc.scalar.activation(
                        r1p[:, cols], zs[1][:], AF.Relu,
                        bias=qv_sb[:, b * HC + 1 : b * HC + 2],
                    )
                else:
                    dve_relu(r1p[:, cols], zs[1][:], 1)
            if b == BC - 1:
                r01p = b3r.tile([128, PW], F16, tag=f"r01b3_{p}")
            else:
                r01p = rsp.tile([128, PW], F16, tag="r01p")
            nc.vector.tensor_tensor(r01p[:], r0p[:], r1p[:], op=ALU.add)
            if b < BC - 1:
                row = b * NP + p
                nc.gpsimd.tensor_reduce(
                    sall[:, row * PW : (row + 1) * PW], r01p[:],
                    axis=mybir.AxisListType.C, op=ALU.add,
                )
            else:
                r01_b3.append(r01p)

        def b3_red(p):
            # psc8 row 2p+s <- sum_h of pair p's half s (sub order)
            for s in range(2):
                j = 2 * p + s
                nc.tensor.matmul(
                    psc8[:], ind_ap(j),
                    r01_b3[p][:, s * SUB : (s + 1) * SUB],
                    start=(p == 0 and s == 0),
                    stop=(p == NP - 1 and s == 1),
                )

        for b in range(BC - 1):
            for p in range(NP):
                do_pair(b, p)

        # b3 main compute; the b0..b2 tail ops are emitted at stream
        # positions where their deps are already satisfied, and the b3
        # reduction matmuls lag one pair behind the fold that feeds them
        do_pair(BC - 1, 0)
        # gather the partition-0 scores strip into [12, PW] via a DRAM
        # bounce (cheap: 48 KB each way, hidden under b3 compute)
        nc.sync.dma_start(sc_d[:], sall[:])
        nc.sync.dma_start(scores12[:], sc_d[:])
        # exp for b0..b2 (ACT stream: lands after b3p0's relus)
        nc.scalar.activation(
            exp12t[:], scores12[:], AF.Exp, accum_out=sums12[:],
        )
        do_pair(BC - 1, 1)
        b3_red(0)
        do_pair(BC - 1, 2)
        b3_red(1)
        do_pair(BC - 1, 3)
        b3_red(2)
        b3_red(3)

        # ---- remaining tail ----
        gs12 = pscp.tile([12, 1], F32, tag="gsx")
        nc.tensor.matmul(gs12[:], c32_sb[0:12, 8:20], sums12[:],
                         start=True, stop=True)
        exp8 = tailp.tile([8, SUB], F32, tag="exp8")
        acc8 = tailp.tile([8, 1], F32, tag="acc8")
        nc.scalar.activation(exp8[:], psc8[:], AF.Exp, accum_out=acc8[:])
        recip12 = tailp.tile([12, 1], F32, tag="recip12")
        nc.vector.reciprocal(recip12[:], gs12[:])
        outs12 = tailp.tile([12, PW], F32, tag="outs12")
        nc.vector.tensor_scalar_mul(outs12[:], exp12t[:], recip12[:])
        nc.sync.dma_start(out_d[0:24], outs12[:])
        gs8 = pscp.tile([8, 1], F32, tag="gsx")
        nc.tensor.matmul(gs8[:], c32_sb[0:8, 24:32], acc8[:],
                         start=True, stop=True)
        recip8 = tailp.tile([8, 1], F32, tag="recip8")
        nc.vector.reciprocal(recip8[:], gs8[:])
        outs8 = tailp.tile([8, SUB], F32, tag="outs8")
        nc.vector.tensor_scalar_mul(outs8[:, 0:256], exp8[:, 0:256], recip8[:])
        nc.sync.dma_start(out_d[24:32, 0:256], outs8[:, 0:256])
        nc.vector.tensor_scalar_mul(outs8[:, 256:SUB], exp8[:, 256:SUB], recip8[:])
        nc.sync.dma_start(out_d[24:32, 256:SUB], outs8[:, 256:SUB])

    nc.compile()
    _CACHE["nc"] = nc
    return nc


def _prep_inputs(hidden, encoder_outputs, attn_w, attn_b, v):
    w1 = attn_w[:, :H]
    w2 = attn_w[:, H:]
    qv_full = (((hidden @ w1.T) + attn_b) * v).astype(np.float32)   # [B, H]
    w2v = (w2 * v[:, None]).astype(np.float32)     # [H(h), H(k)]
    w2v_T = np.ascontiguousarray(w2v.T)            # [k, h]

    w16 = np.zeros((128, KC * HC * 128), dtype=np.float16)
    for kc in range(KC):
        for hc in range(HC):
            off = (kc * HC + hc) * 128
            w16[:, off : off + 128] = w2v_T[
                kc * 128 : (kc + 1) * 128, hc * 128 : (hc + 1) * 128
            ].astype(np.float16)

    c16 = np.zeros((128, 512 + 64), dtype=np.float16)
    c16[:, 0:512] = w16
    for j in range(8):
        c16[:, 512 + 8 * j + j] = 1.0

    enc16 = encoder_outputs.astype(np.float16)     # [T, B, H]
    in_maps = []
    for c in range(NCORES):
        bs = c * BC
        enc_c = np.ascontiguousarray(
            enc16[:, bs : bs + BC, :].transpose(1, 2, 0)
        ).reshape(BC, KC, 128, T)
        qv_c = np.ascontiguousarray(
            qv_full[bs : bs + BC].reshape(BC, HC, 128).transpose(2, 0, 1)
        ).reshape(128, BC * HC)
        c32 = np.zeros((128, 32), dtype=np.float32)
        c32[:, 0:8] = qv_c
        for bb in range(3):
            c32[bb * 4 : (bb + 1) * 4, 8 + bb * 4 : 8 + (bb + 1) * 4] = 1.0
        c32[0:8, 24:32] = 1.0
        in_maps.append(
            {"enc": enc_c, "c16": c16, "c32": c32}
        )
    return in_maps


def kernel(hidden, encoder_outputs, attn_w, attn_b, v):
    global LAST_RESULTS
    nc = _build()
    in_maps = _prep_inputs(
        np.asarray(hidden, dtype=np.float32),
        np.asarray(encoder_outputs, dtype=np.float32),
        np.asarray(attn_w, dtype=np.float32),
        np.asarray(attn_b, dtype=np.float32),
        np.asarray(v, dtype=np.float32),
    )
    res = run_bass_kernel_spmd(nc, in_maps, list(range(NCORES)))
    LAST_RESULTS = res
    out = np.empty((B, 1, T), dtype=np.float32)
    for c in range(NCORES):
        out[c * BC : (c + 1) * BC, 0, :] = (
            res.results[c]["scores"].reshape(BC, T)
        )
    return out


# revision 57
# speedup vs baseline: 1.5158x; 1.0152x over previous
"""HardAttention Bass kernel for 8 TRN2 NeuronCores.

reference math (B=32, T=4096, H=256):
  energy[b,t,h] = relu( sum_k cat(hidden,enc)[b,t,k] * attn_w[h,k] + attn_b[h] )
  scores[b,t]   = sum_h energy[b,t,h] * v[h]
  out           = softmax(scores, axis=t)[:, None, :]

Device strategy (data-parallel over B, 4 batches/core):
  * split attn_w into W1 (hidden half) and W2 (encoder half)
  * fold v into W2 and into the per-batch bias q = hidden@W1.T + attn_b
    (valid because v >= 0: relu(x)*v == relu(x*v))
  * enc streamed as fp16 [k, t] tiles (k on partitions); per (b, 512-col
    sub): z[h,t] = W2v-tiles @ enc-tiles (fp16 matmuls, fp32 psum)
  * relu+bias fused per h-chunk: ACT activation / DVE tensor_scalar,
    writing fp16 r tiles; DVE folds the two h-chunks (fp16 2x add);
    Pool (gpsimd) reduces the 128 h-partitions -> scores row [1, 512]
  * scores live as [32, 512] (row = b*8+sub): exp+accum on ACT in one
    [32,512] op, per-b sums and reciprocal broadcast via tiny PE
    indicator matmuls, final scale on DVE (2x sbuf mode), one DMA out
Host reassembles [32,512] -> [4, 4096] per core (pure reshape).
"""

from contextlib import ExitStack

import numpy as np

import concourse.bass as bass
import concourse.tile as tile
from concourse import bacc, mybir
from concourse.bass_utils import run_bass_kernel_spmd

B, T, H = 32, 4096, 256
NCORES = 8
BC = B // NCORES            # 4 batches per core
KC = H // 128               # 2 k-chunks
HC = H // 128               # 2 h-chunks
SUB = 512                   # t columns per reduction unit
NSUB = T // SUB             # 8 subs per batch
DMAW = 2048                 # t columns per enc DMA
NDMA = T // DMAW            # 2 DMA halves per (b, kc)

F32 = mybir.dt.float32
F16 = mybir.dt.float16

_CACHE = {}
LAST_RESULTS = None


def _build():
    if "nc" in _CACHE:
        return _CACHE["nc"]

    nc = bacc.Bacc(None, target_bir_lowering=False)
    enc_d = nc.dram_tensor("enc", [BC, KC, 128, T], F16, kind="ExternalInput")
    # c16: cols 0:512 = w2v lhsT tiles; cols 512:576 = ind64 (8 blocks of
    # [128,8] with column j of block j all-ones, for b3 PE reductions)
    c16_d = nc.dram_tensor("c16", [128, 512 + 64], F16, kind="ExternalInput")
    # c32: cols 0:8 = qv (per-(b,hc) bias columns); cols 8:24 rows 0:12 = bc
    # (block-ones(4) over b0..b2 rows, and 8x4 all-ones for the b3 total)
    c32_d = nc.dram_tensor("c32", [128, 32], F32, kind="ExternalInput")
    out_d = nc.dram_tensor("scores", [32, SUB], F32, kind="ExternalOutput")
    sc_d = nc.dram_tensor("sc_scratch", [12, 1024], F32, kind="Internal")

    AF = mybir.ActivationFunctionType
    ALU = mybir.AluOpType

    NP = NSUB // 2          # 4 sub-pairs per batch
    PW = 2 * SUB            # 1024 columns per pair

    # engine for each hc1 relu by (b, sub); hc0 relus always run on ACT
    HC1 = {}
    for b in range(BC):
        for s in range(NSUB):
            HC1[(b, s)] = "dve"
    for b in range(BC):
        for s in [0, 4]:
            HC1[(b, s)] = "act"

    with tile.TileContext(nc) as tc, ExitStack() as ctx:
        const = ctx.enter_context(tc.tile_pool(name="const", bufs=1))
        encp = ctx.enter_context(tc.tile_pool(name="encp", bufs=1))
        zp = ctx.enter_context(tc.tile_pool(name="zp", bufs=6, space="PSUM"))
        rsp = ctx.enter_context(tc.tile_pool(name="rsp", bufs=6))
        b3r = ctx.enter_context(tc.tile_pool(name="b3r", bufs=1))
        tailp = ctx.enter_context(tc.tile_pool(name="tail", bufs=1))
        pscp = ctx.enter_context(tc.tile_pool(name="pscp", bufs=1, space="PSUM"))

        # fp16 consts in one DMA on the SP queue ahead of the enc stream;
        # fp32 consts follow the first small enc tiles
        c16_sb = const.tile([128, 512 + 64], F16, tag="c16")
        nc.sync.dma_start(c16_sb[:], c16_d[:])
        c32_sb = const.tile([128, 32], F32, tag="c32")
        w16_sb = c16_sb
        qv_sb = c32_sb


        def w2v_ap(kc, hc):
            off = (kc * HC + hc) * 128
            return w16_sb[:, off : off + 128]

        def ind_ap(j):
            return c16_sb[:, 512 + 8 * j : 512 + 8 * j + 8]

        def bc_ap(rows, cols):
            return c32_sb[rows, 8 + cols.start : 8 + cols.stop]

        # stream in all enc tiles (resident: 8.4 MB total). The first two
        # (b0, half0) tiles are split [0:512]+[512:2048] so PE's first pair
        # only waits on two small transfers.
        enc_t = {}
        first = {}
        for kc in range(KC):
            ft = encp.tile([128, SUB], F16, tag=f"enc_f_{kc}")
            nc.gpsimd.dma_start(ft[:], enc_d[0, kc][:, 0:SUB])
            first[kc] = ft
        nc.sync.dma_start(c32_sb[:], c32_d[:])

        HW = DMAW // 2
        for b in range(BC):
            for half in range(NDMA):
                for kc in range(KC):
                    et = encp.tile([128, DMAW], F16, tag=f"enc_{b}_{kc}_{half}")
                    enc_t[(b, kc, half)] = et
                if b == 0:
                    # finer chunks for the first batch keep PE fed while it
                    # chases the incoming stream
                    for kc in range(KC):
                        for q in range(2):
                            lo = half * DMAW + q * HW
                            nc.sync.dma_start(
                                enc_t[(b, kc, half)][:, q * HW : (q + 1) * HW],
                                enc_d[b, kc][:, lo : lo + HW],
                            )
                else:
                    for kc in range(KC):
                        nc.sync.dma_start(
                            enc_t[(b, kc, half)][:],
                            enc_d[b, kc][:, half * DMAW : (half + 1) * DMAW],
                        )

        scores12 = tailp.tile([12, PW], F32, tag="scores12")
        sall = tailp.tile([1, 12 * PW], F32, tag="sall")
        exp12t = tailp.tile([12, PW], F32, tag="exp12t")
        sums12 = tailp.tile([12, 1], F32, tag="sums12")
        psc8 = pscp.tile([8, SUB], F32, tag="psc8")


        r01_b3 = []
        _rows_done = set()

        def do_pair(b, p):
            r0p = rsp.tile([128, PW], F16, tag="r0p")
            r1p = rsp.tile([128, PW], F16, tag="r1p")
            last = b == BC - 1 and p == NP - 1

            def dve_relu(dst, zsrc, hc):
                nc.vector.tensor_scalar(
                    dst, zsrc,
                    scalar1=qv_sb[:, b * HC + hc : b * HC + hc + 1],
                    scalar2=0.0,
                    op0=ALU.add,
                    op1=ALU.max,
                )

            for s in range(2):
                sub = 2 * p + s
                half, col = divmod(sub * SUB, DMAW)
                zs = []
                for hc in range(HC):
                    z = zp.tile([128, SUB], F32, tag="z")
                    for kc in range(KC):
                        if b == 0 and sub == 0:
                            rhs = first[kc][:]
                        else:
                            rhs = enc_t[(b, kc, half)][:, col : col + SUB]
                        nc.tensor.matmul(
                            z[:],
                            w2v_ap(kc, hc),
                            rhs,
                            start=(kc == 0),
                            stop=(kc == KC - 1),
                        )
                    zs.append(z)
                cols = slice(s * SUB, (s + 1) * SUB)
                nc.scalar.activation(
                    r0p[:, cols], zs[0][:], AF.Relu,
                    bias=qv_sb[:, b * HC : b * HC + 1],
                )
                eng = HC1.get((b, sub), "dve")
                if eng == "act":
                    nc.scalar.activation(
                        r1p[:, cols], zs[1][:], AF.Relu,
                        bias=qv_sb[:, b * HC + 1 : b * HC + 2],
                    )
                elif eng == "pool":
                    nc.gpsimd.tensor_scalar(
                        r1p[:, cols], zs[1][:],
                        scalar1=qv_sb[:, b * HC + 1 : b * HC + 2],
                        scalar2=0.0,
                        op0=ALU.add,
                        op1=ALU.max,
                    )
                else:
                    dve_relu(r1p[:, cols], zs[1][:], 1)
            if b == BC - 1:
                r01p = b3r.tile([128, PW], F16, tag=f"r01b3_{p}")
            else:
                r01p = rsp.tile([128, PW], F16, tag="r01p")
            nc.vector.tensor_tensor(r01p[:], r0p[:], r1p[:], op=ALU.add)
            if b < BC - 1:
                row = b * NP + p
                nc.gpsimd.tensor_reduce(
                    sall[:, row * PW : (row + 1) * PW], r01p[:],
                    axis=mybir.AxisListType.C, op=ALU.add,
                )
            else:
                r01_b3.append(r01p)

        def b3_red(p):
            # psc8 row 2p+s <- sum_h of pair p's half s (sub order)
            for s in range(2):
                j = 2 * p + s
                nc.tensor.matmul(
                    psc8[:], ind_ap(j),
                    r01_b3[p][:, s * SUB : (s + 1) * SUB],
                    start=(p == 0 and s == 0),
                    stop=(p == NP - 1 and s == 1),
                )

        for b in range(BC - 1):
            for p in range(NP):
                do_pair(b, p)

        # b3 main compute; the b0..b2 tail ops are emitted at stream
        # positions where their deps are already satisfied, and the b3
        # reduction matmuls lag one pair behind the fold that feeds them
        do_pair(BC - 1, 0)
        # gather the partition-0 scores strip into [12, PW] via a DRAM
        # bounce (cheap: 48 KB each way, hidden under b3 compute)
        nc.sync.dma_start(sc_d[:], sall[:])
        nc.sync.dma_start(scores12[:], sc_d[:])
        # exp for b0..b2 (ACT stream: lands after b3p0's relus)
        nc.scalar.activation(
            exp12t[:], scores12[:], AF.Exp, accum_out=sums12[:],
        )
        do_pair(BC - 1, 1)
        b3_red(0)
        do_pair(BC - 1, 2)
        b3_red(1)
        # b0..b2 totals: the tiny matmul goes here so PE reaches it long
        # before its last main matmul (exp12's accum is ready by now), and
        # the whole 12-row tail (recip/scale/DMA) hides under b3 compute
        gs12 = pscp.tile([12, 1], F32, tag="gsx")
        nc.tensor.matmul(gs12[:], c32_sb[0:12, 8:20], sums12[:],
                         start=True, stop=True)
        recip12 = tailp.tile([12, 1], F32, tag="recip12")
        nc.vector.reciprocal(recip12[:], gs12[:])
        outs12 = tailp.tile([12, PW], F32, tag="outs12")
        nc.vector.tensor_scalar_mul(outs12[:], exp12t[:], recip12[:])
        nc.sync.dma_start(out_d[0:24], outs12[:])
        do_pair(BC - 1, 3)
        b3_red(2)
        b3_red(3)

        # ---- remaining tail ----
        exp8 = tailp.tile([8, SUB], F32, tag="exp8")
        acc8 = tailp.tile([8, 1], F32, tag="acc8")
        nc.scalar.activation(exp8[:], psc8[:], AF.Exp, accum_out=acc8[:])
        gs8 = pscp.tile([8, 1], F32, tag="gsx")
        nc.tensor.matmul(gs8[:], c32_sb[0:8, 24:32], acc8[:],
                         start=True, stop=True)
        recip8 = tailp.tile([8, 1], F32, tag="recip8")
        nc.vector.reciprocal(recip8[:], gs8[:])
        outs8 = tailp.tile([8, SUB], F32, tag="outs8")
        nc.vector.tensor_scalar_mul(outs8[:], exp8[:], recip8[:])
        nc.sync.dma_start(out_d[24:32], outs8[:])

    nc.compile()
    _CACHE["nc"] = nc
    return nc


def _prep_inputs(hidden, encoder_outputs, attn_w, attn_b, v):
    w1 = attn_w[:, :H]
    w2 = attn_w[:, H:]
    qv_full = (((hidden @ w1.T) + attn_b) * v).astype(np.float32)   # [B, H]
    w2v = (w2 * v[:, None]).astype(np.float32)     # [H(h), H(k)]
    w2v_T = np.ascontiguousarray(w2v.T)            # [k, h]

    w16 = np.zeros((128, KC * HC * 128), dtype=np.float16)
    for kc in range(KC):
        for hc in range(HC):
            off = (kc * HC + hc) * 128
            w16[:, off : off + 128] = w2v_T[
                kc * 128 : (kc + 1) * 128, hc * 128 : (hc + 1) * 128
            ].astype(np.float16)

    c16 = np.zeros((128, 512 + 64), dtype=np.float16)
    c16[:, 0:512] = w16
    for j in range(8):
        c16[:, 512 + 8 * j + j] = 1.0

    enc16 = encoder_outputs.astype(np.float16)     # [T, B, H]
    in_maps = []
    for c in range(NCORES):
        bs = c * BC
        enc_c = np.ascontiguousarray(
            enc16[:, bs : bs + BC, :].transpose(1, 2, 0)
        ).reshape(BC, KC, 128, T)
        qv_c = np.ascontiguousarray(
            qv_full[bs : bs + BC].reshape(BC, HC, 128).transpose(2, 0, 1)
        ).reshape(128, BC * HC)
        c32 = np.zeros((128, 32), dtype=np.float32)
        c32[:, 0:8] = qv_c
        for bb in range(3):
            c32[bb * 4 : (bb + 1) * 4, 8 + bb * 4 : 8 + (bb + 1) * 4] = 1.0
        c32[0:8, 24:32] = 1.0
        in_maps.append(
            {"enc": enc_c, "c16": c16, "c32": c32}
        )
    return in_maps


def kernel(hidden, encoder_outputs, attn_w, attn_b, v):
    global LAST_RESULTS
    nc = _build()
    in_maps = _prep_inputs(
        np.asarray(hidden, dtype=np.float32),
        np.asarray(encoder_outputs, dtype=np.float32),
        np.asarray(attn_w, dtype=np.float32),
        np.asarray(attn_b, dtype=np.float32),
        np.asarray(v, dtype=np.float32),
    )
    res = run_bass_kernel_spmd(nc, in_maps, list(range(NCORES)))
    LAST_RESULTS = res
    out = np.empty((B, 1, T), dtype=np.float32)
    for c in range(NCORES):
        out[c * BC : (c + 1) * BC, 0, :] = (
            res.results[c]["scores"].reshape(BC, T)
        )
    return out
